# revision 1
# baseline (speedup 1.0000x reference)
"""Trainium2 Bass kernel for nn_AdaptativeGCN (gnn_message_passing).

Computation (reference):
    sec   = relu(A @ (X Ws1) + X Ws2 + bs)                 [N, 32]
    S     = [sec | P]                                      [N, 96]
    msec  = A @ (S Wm2a) + S Wm2b + bm2                    [N, 7]
    M     = [X | P]                                        [N, 192]
    main  = A @ (M Wma) + M Wmb + bm                       [N, 7]
    out   = softmax(0.5*(v2*main + v1*msec), axis=-1)      [N, 7]

Algebraic restructuring (v3):
    All X-mediated A-terms are reassociated through AXt = (A_loc @ X)^T
    [128, RL], computed ONCE by streaming the fp8 A-shard through the PE
    with X row-tiles as DoubleRow stationaries:
      sec_loc  = relu(Ws1^T AXt + Ws2^T Xloc^T + bs)       (projections)
      mainx    = 0.5 v2 (Wma_x^T AXt + Wmb_x^T Xloc^T)     (local X part)
    The only cross-core term is Gc = [sec|P] @ folded-weights [N, 7]
    (one fused [96,cw]-stationary projection per 128-row group): one
    small fp8 AllGather, then one more A-stream pass (2b) contracts it.
    2b runs chunk-major so each 512-col chunk's epilogue (add, PE
    transpose) overlaps the remaining chunks' matmuls; clock-keeper
    matmuls fill the AllGather wait so HAM holds full PE clock. Local
    additive terms L and biases fold into the epilogue add before the
    transpose+softmax.

Distribution: row-shard A over 8 cores (1250 rows each). Host uploads
A[rows_c,:].T as fp8-e4m3 [10000, 1250] per core, streamed over BOTH
HWDGE queues (sync+scalar) and kept SBUF-resident for pass 2b.
"""

import sys
import types

import numpy as np


def _install_ntff_hook():
    """run_bass_kernel_spmd(trace=True) under axon needs antenv.axon_hooks,
    which the agent image lacks; register the ctypes-based hook ourselves."""
    try:
        from antenv.axon_hooks import get_axon_ntff_profile_hook  # noqa: F401
        return
    except ImportError:
        pass
    try:
        from trn_agent_boot.trn_boot import _ntff_profile_via_ctypes
        hook = _ntff_profile_via_ctypes('/opt/axon/libaxon_pjrt.so')
    except Exception:
        hook = None
    mod = types.ModuleType('antenv.axon_hooks')
    mod.get_axon_ntff_profile_hook = lambda: hook
    mod.set_axon_ntff_profile_hook = lambda h: None
    sys.modules['antenv.axon_hooks'] = mod


_install_ntff_hook()
if '/opt/trn_rl_repo' not in sys.path:
    sys.path.insert(0, '/opt/trn_rl_repo')

import os  # noqa: E402

import ml_dtypes  # noqa: E402
import concourse.bacc as bacc  # noqa: E402
import concourse.mybir as mybir  # noqa: E402
from concourse import masks, tile  # noqa: E402
from concourse.bass_utils import run_bass_kernel_spmd  # noqa: E402

BF16 = ml_dtypes.bfloat16
FP8 = ml_dtypes.float8_e4m3
NCORES = 8
N = 10000
F_T, F_P = 128, 64
SEC, MC = 32, 7
RL = N // NCORES            # local rows per core = 1250
RLP = 1280                  # padded SBUF pitch per AT k-tile (16B-aligned)
NT = (N + 127) // 128       # k-tiles over the contraction dim = 79
KW = [128] * (NT - 1) + [N - 128 * (NT - 1)]          # last = 16
CHUNKS = [(0, 512), (512, 512), (1024, RL - 1024)]    # free-dim chunks of 1250
RC = [(i * 128, min(128, RL - i * 128)) for i in range((RL + 127) // 128)]
NPAIR = (NT - 1) // 2       # 39 DoubleRow k-tile pairs; tile 78 done plain
GCP = 16                    # padded Gc pitch per k-tile (DoubleRow: step%16==0)
DMA_GROUP = int(os.environ.get("DMAGRP", "4"))  # AT k-tiles per batched DMA
JUNK2B = int(os.environ.get("JUNK2B", "48"))    # clock-keeper MMs in CC window
VJUNK = int(os.environ.get("VJUNK", "0"))       # DVE clock-keepers (no effect:
                                                # HAM is per-engine; measured)

_compiled = None


def _build():
    f32 = mybir.dt.float32
    bf16 = mybir.dt.bfloat16
    fp8 = mybir.dt.float8e4

    nc = bacc.Bacc("TRN2", target_bir_lowering=False, debug=False,
                   num_devices=NCORES)

    # ---- per-core inputs ----
    at_e = nc.dram_tensor("at", [N, RL], fp8, kind="ExternalInput").ap()
    xn_e = nc.dram_tensor("xn", [128, NT * 128], fp8,
                          kind="ExternalInput").ap()
    xtl_e = nc.dram_tensor("xtl", [F_T, RL], bf16, kind="ExternalInput").ap()
    pt_e = nc.dram_tensor("pt", [F_P, RL], bf16, kind="ExternalInput").ap()
    ws1_e = nc.dram_tensor("ws1", [F_T, SEC], bf16, kind="ExternalInput").ap()
    ws2_e = nc.dram_tensor("ws2", [F_T, SEC], bf16, kind="ExternalInput").ap()
    bs_e = nc.dram_tensor("bs", [SEC, 1], f32, kind="ExternalInput").ap()
    wgsp_e = nc.dram_tensor("wgsp", [SEC + F_P, MC], bf16,
                            kind="ExternalInput").ap()
    was_e = nc.dram_tensor("was", [SEC, MC], bf16, kind="ExternalInput").ap()
    wxa_e = nc.dram_tensor("wxa", [F_T, MC], bf16, kind="ExternalInput").ap()
    wxb_e = nc.dram_tensor("wxb", [F_T, MC], bf16, kind="ExternalInput").ap()
    wap_e = nc.dram_tensor("wap", [F_P, MC], bf16, kind="ExternalInput").ap()
    blb_e = nc.dram_tensor("blb", [MC, 512], f32, kind="ExternalInput").ap()
    out_e = nc.dram_tensor("out", [RL, MC], f32, kind="ExternalOutput").ap()

    with tile.TileContext(nc) as tc:
        with (
            tc.tile_pool(name="const", bufs=1) as cp,
            tc.tile_pool(name="big", bufs=1) as bigp,
            tc.tile_pool(name="work", bufs=1) as wp,
            tc.tile_pool(name="psum", bufs=1, space="PSUM") as pp,
            tc.tile_pool(name="dram", bufs=1, space="DRAM") as dp,
        ):
            # ---- constants / persistent tiles (small DMAs on gpsimd,
            # keeping the HWDGE queues free for the AT stream) ----
            ws1_s = cp.tile([F_T, SEC], bf16)
            ws2_s = cp.tile([F_T, SEC], bf16)
            bs_s = cp.tile([SEC, 1], f32)
            wgsp_s = cp.tile([SEC + F_P, MC], bf16)
            was_s = cp.tile([SEC, MC], bf16)
            wxa_s = cp.tile([F_T, MC], bf16)
            wxb_s = cp.tile([F_T, MC], bf16)
            wap_s = cp.tile([F_P, MC], bf16)
            blb_s = cp.tile([MC, 512], f32)
            eye_s = cp.tile([MC, MC], f32)
            xtl_s = cp.tile([F_T, RL], bf16)
            pt_s = cp.tile([F_P, RL], bf16)
            xn_s = bigp.tile([128, NT * 128], fp8, name="xnfull")
            # [sec | P]^T stacked: rows 0:32 = sec (relu output), rows
            # 32:96 = P^T — one stationary for the fused Gc projection
            spt_s = bigp.tile([SEC + F_P, RL], bf16, name="spt")
            # xn rides the gpsimd software queue (chunked, head first) so
            # BOTH HWDGE queues carry nothing but the AT stream
            nc.gpsimd.dma_start(xn_s[:, 0:6 * 128], xn_e[:, 0:6 * 128])
            nc.gpsimd.dma_start(xn_s[:, 6 * 128:40 * 128],
                                xn_e[:, 6 * 128:40 * 128])
            nc.gpsimd.dma_start(xn_s[:, 40 * 128:], xn_e[:, 40 * 128:])
            for dst, src in [(ws1_s, ws1_e), (wxa_s, wxa_e), (wxb_s, wxb_e),
                             (ws2_s, ws2_e), (bs_s, bs_e),
                             (wgsp_s, wgsp_e),
                             (was_s, was_e), (wap_s, wap_e),
                             (blb_s, blb_e), (xtl_s, xtl_e), (pt_s, pt_e)]:
                nc.gpsimd.dma_start(dst[:], src[:])
            nc.gpsimd.dma_start(spt_s[SEC:SEC + F_P, :], pt_e[:])
            masks.make_identity(nc, eye_s[:])

            axs = bigp.tile([128, RL], bf16, name="axs")     # (A X)^T bf16
            mainx = bigp.tile([MC, RL], f32, name="mainx")
            gcf = bigp.tile([128, NT * GCP], fp8, name="gcf")
            combT = bigp.tile([MC, RL], f32, name="combT")
            at_s = bigp.tile([128, NT * RLP], fp8, name="atcache")
            at3 = at_s[:].rearrange("p (k i) -> p k i", i=RLP)
            xn3 = xn_s[:].rearrange("p (k f) -> p k f", f=128)
            gcv = gcf[:].rearrange("p (k c) -> p k c", c=GCP)
            nc.gpsimd.memset(gcf[:], 0.0)

            # ---- AT stream: batched DMAs split across BOTH HWDGE queues
            # (sync + scalar) so the 12.5 MiB shard lands at aggregate HBM
            # rate. One junk matmul rides each group so the PE never idles
            # a full HAM MID window during the DMA phase. ----
            # early PE warm-up on xn (lands ~10us, well before the first
            # AT group at ~19us) so HAM ramps the clock before pass 1
            for wi in range(6):
                pxw = pp.tile([128, 512], f32, tag="warm", bufs=1,
                              name=f"pxw{wi}")
                nc.tensor.matmul(pxw[:, :], xn_s[:, wi * 128:wi * 128 + 128],
                                 xn_s[:, 0:512], start=True, stop=True)

            _dma_engs = [nc.scalar, nc.sync]
            for gi, g0 in enumerate(range(0, NT - 1, DMA_GROUP)):
                g1_ = min(g0 + DMA_GROUP, NT - 1)
                _dma_engs[gi % len(_dma_engs)].dma_start(
                    at3[:, g0:g1_, 0:RL],
                    at_e[g0 * 128:g1_ * 128, :].rearrange(
                        "(g p) i -> p g i", p=128))
                if os.environ.get("NOWARM") != "1":
                    pw = pp.tile([128, 512], f32, tag="warm", bufs=1,
                                 name=f"pw{gi}")
                    nc.tensor.matmul(pw[:16, :16], at3[:, g0, 0:16],
                                     at3[:, g0, 0:16], start=True, stop=True)
            nc.scalar.dma_start(at3[:KW[NT - 1], NT - 1, 0:RL],
                                at_e[(NT - 1) * 128:N, :])

            # ---- pass 1: axt[ci] = ((A_loc @ X)^T) chunk, fp8 DoubleRow
            # with X row-tiles stationary. The Ws2 half of the sec
            # projection runs into its own psum banks DURING the stream. ----
            kl, kwl = NT - 1, KW[NT - 1]
            axt = [pp.tile([128, 512], f32, tag="acc", bufs=3, name=f"ax{i}")
                   for i in range(3)]
            for j in range(NPAIR):
                for ci, (off, w) in enumerate(CHUNKS):
                    nc.tensor.matmul(axt[ci][:, :w], xn3[:, 2 * j:2 * j + 2, :],
                                     at3[:, 2 * j:2 * j + 2, off:off + w],
                                     start=(j == 0), stop=False,
                                     perf_mode=mybir.MatmulPerfMode.DoubleRow)
            for ci, (off, w) in enumerate(CHUNKS):
                nc.tensor.matmul(axt[ci][:, :w], xn3[:kwl, kl, :],
                                 at3[:kwl, kl, off:off + w],
                                 start=False, stop=True)
            for ci, (off, w) in enumerate(CHUNKS):
                nc.vector.tensor_copy(axs[:, off:off + w], axt[ci][:, :w])

            # ---- sec pre-act: Ws1^T (A X)^T + Ws2^T Xloc^T ----
            ps_s = [pp.tile([SEC, 512], f32, tag="acc", bufs=3,
                            name=f"ps{i}") for i in range(3)]
            for ci, (off, w) in enumerate(CHUNKS):
                nc.tensor.matmul(ps_s[ci][:, :w], ws1_s[:], axs[:, off:off + w],
                                 start=True, stop=False)
                nc.tensor.matmul(ps_s[ci][:, :w], ws2_s[:],
                                 xtl_s[:, off:off + w],
                                 start=False, stop=True)
            # ---- sec^T = relu(ps_s + bs), into rows 0:32 of [sec|P]^T ----
            for ci, (off, w) in enumerate(CHUNKS):
                nc.scalar.activation(spt_s[0:SEC, off:off + w],
                                     ps_s[ci][:, :w],
                                     mybir.ActivationFunctionType.Relu,
                                     bias=bs_s[:, :])

            # ---- Gc_loc (natural [RL, 7], fp8) -> bounce -> AllGather ----
            gc_bounce = dp.tile([RL, MC], fp8, name="gc_bounce")
            gc_gather = dp.tile([N, MC], fp8, addr_space="Shared",
                                name="gc_gather")
            gcl = wp.tile([128, len(RC) * MC], fp8, name="gcl")
            gclv = gcl[:].rearrange("p (g c) -> p g c", c=MC)
            for ri, (o2, cw) in enumerate(RC):
                pgc = pp.tile([128, 70], f32, tag="small", bufs=3,
                              name=f"pgc{ri}")
                nc.tensor.matmul(pgc[:cw, :MC], spt_s[:, o2:o2 + cw],
                                 wgsp_s[:], start=True, stop=True)
                nc.vector.tensor_copy(gclv[:cw, ri, :], pgc[:cw, :MC])
            # staged bounce: first half ships while later row-groups compute
            nc.sync.dma_start(
                gc_bounce[0:640, :].rearrange("(g p) c -> p g c", p=128),
                gclv[:, 0:5, :])
            nc.sync.dma_start(
                gc_bounce[640:1152, :].rearrange("(g p) c -> p g c", p=128),
                gclv[:, 5:9, :])
            nc.sync.dma_start(gc_bounce[1152:RL, :], gclv[:98, 9, :])
            nc.gpsimd.collective_compute(
                "AllGather", mybir.AluOpType.bypass,
                ins=[gc_bounce[:].opt()], outs=[gc_gather[:].opt()],
                replica_groups=[list(range(NCORES))],
            )

            # ---- local additive terms into psum_main (reuses acc slots) ----
            ps_m = [pp.tile([SEC, 512], f32, tag="acc", bufs=3, name=f"pm{i}")
                    for i in range(3)]
            for ci, (off, w) in enumerate(CHUNKS):
                nc.tensor.matmul(ps_m[ci][:MC, :w], was_s[:],
                                 spt_s[0:SEC, off:off + w],
                                 start=True, stop=False)
                nc.tensor.matmul(ps_m[ci][:MC, :w], wap_s[:],
                                 pt_s[:, off:off + w], start=False, stop=False)

            # ---- main-X local term (only needed at the epilogue; emitted
            # after the collective issue so it fills the wait window) ----
            px = [pp.tile([MC, 512], f32, tag="warm", bufs=1, name=f"px{i}")
                  for i in range(3)]
            for ci, (off, w) in enumerate(CHUNKS):
                nc.tensor.matmul(px[ci][:, :w], wxa_s[:], axs[:, off:off + w],
                                 start=True, stop=False)
                nc.tensor.matmul(px[ci][:, :w], wxb_s[:], xtl_s[:, off:off + w],
                                 start=False, stop=True)
                # mainx = px + bl (bl pre-broadcast in blb); frees the slot
                nc.vector.tensor_add(mainx[:, off:off + w], px[ci][:, :w],
                                     blb_s[:, :w])

            # ---- clock-keeper: junk matmuls that fill the AllGather wait
            # so HAM keeps the PE at full clock for pass 2b. Stationary
            # reads `sect` so the scheduler places them after pass 1; the
            # single psum buf serializes the chain. ----
            for ji in range(JUNK2B):
                pj = pp.tile([128, 512], f32, tag="warm", bufs=1,
                             name=f"pj{ji}")
                nc.tensor.matmul(
                    pj[:, :],
                    spt_s[0:SEC, (ji % 9) * 128:(ji % 9) * 128 + 128],
                    xtl_s[:SEC, 0:512],
                    start=True, stop=True)

            # ---- DVE clock-keepers: copies chained on one scratch tile
            # fill the wait window on the Vector engine (first one reads
            # the relu output to anchor placement after pass 1) ----
            if VJUNK:
                vj = wp.tile([128, 512], bf16, name="vjunk")
            for vi in range(VJUNK):
                src = (spt_s[0:SEC, 0:512] if vi == 0
                       else at3[:, vi % (NT - 1), 0:512])
                nc.vector.tensor_copy(vj[:SEC if vi == 0 else 128, :],
                                      src)

            # ---- load gathered Gc into SBUF k-tiles (chunked DMAs so
            # pass 2b can start as soon as the first k-tiles land) ----
            GCHUNK = 10
            for c0 in range(0, NT - 1, GCHUNK):
                c1 = min(c0 + GCHUNK, NT - 1)
                nc.sync.dma_start(
                    gcv[:, c0:c1, 0:MC],
                    gc_gather[c0 * 128:c1 * 128, :].rearrange(
                        "(k p) c -> p k c", p=128))
            nc.sync.dma_start(gcv[:KW[NT - 1], NT - 1, 0:MC],
                              gc_gather[(NT - 1) * 128:N, :])

            # ---- pass 2b: += (A @ Gc)_loc^T, AT straight from SBUF.
            # Chunk-major order: chunk ci's accumulation STOPS before the
            # later chunks finish, so its combT add + transposes overlap
            # the remaining 2b matmuls. ----
            ptl = pp.tile([128, 70], f32, tag="ptile", bufs=1, name="ptl")
            for ci, (off, w) in enumerate(CHUNKS):
                for j in range(NPAIR):
                    nc.tensor.matmul(ps_m[ci][:GCP, :w],
                                     gcv[:, 2 * j:2 * j + 2, :],
                                     at3[:, 2 * j:2 * j + 2, off:off + w],
                                     start=False, stop=False,
                                     perf_mode=mybir.MatmulPerfMode.DoubleRow)
                nc.tensor.matmul(ps_m[ci][:MC, :w],
                                 gcf[:kwl, kl * GCP:kl * GCP + MC],
                                 at3[:kwl, kl, off:off + w],
                                 start=False, stop=True)
                nc.vector.tensor_add(combT[:, off:off + w], ps_m[ci][:MC, :w],
                                     mainx[:, off:off + w])
                for ri, (o2, cw) in enumerate(RC):
                    if off <= o2 < off + w:
                        nc.tensor.transpose(ptl[:cw, ri * MC:(ri + 1) * MC],
                                            combT[:, o2:o2 + cw], eye_s[:])
            ex = wp.tile([128, 70], f32, name="ex")
            nrc = len(RC)
            nc.scalar.activation(ex[:, :], ptl[:, :],
                                 mybir.ActivationFunctionType.Exp)
            sm = wp.tile([128, nrc], f32, name="sm")
            nc.vector.tensor_reduce(
                sm[:, :], ex[:].rearrange("p (g c) -> p g c", c=MC),
                axis=mybir.AxisListType.X, op=mybir.AluOpType.add)
            rcp = wp.tile([128, nrc], f32, name="rcp")
            nc.vector.reciprocal(rcp[:, :], sm[:, :])
            ot = wp.tile([128, 70], f32, name="ot")
            nc.vector.tensor_mul(
                ot[:].rearrange("p (g c) -> p g c", c=MC),
                ex[:].rearrange("p (g c) -> p g c", c=MC),
                rcp[:].broadcast_to([128, nrc, MC]))
            nc.sync.dma_start(
                out_e[0:1152, :].rearrange("(g p) c -> p g c", p=128),
                ot[:].rearrange("p (g c) -> p g c", c=MC)[:, 0:9, :])
            nc.sync.dma_start(out_e[1152:RL, :], ot[:98, 63:70])

    nc.compile()
    return nc


def _get_compiled():
    global _compiled
    if _compiled is None:
        _compiled = _build()
    return _compiled


def kernel(temporal_features, A, path_features,
           Ws1, Ws2, bs, Wm2a, Wm2b, bm2, Wma, Wmb, bm, v1, v2,
           trace=False, tmpdir=None, trace_cores=None):
    nc = _get_compiled()

    X = np.asarray(temporal_features, np.float32)
    A = np.asarray(A, np.float32)
    P = np.asarray(path_features, np.float32)
    v1 = np.float32(v1)
    v2 = np.float32(v2)

    A8 = A.astype(FP8)
    xtf = np.ascontiguousarray(X.T)                        # [128, N] f32
    ptf = np.ascontiguousarray(P.T).astype(BF16)           # [64, N]
    # X row-k-tiles, node dim on partitions: xn[p, k*128+f] = X[k*128+p, f]
    Xpad = np.zeros((NT * 128, F_T), np.float32)
    Xpad[:N] = X
    xn = np.ascontiguousarray(
        Xpad.reshape(NT, 128, F_T).transpose(1, 0, 2).reshape(128, NT * F_T)
    ).astype(FP8)

    ws1 = np.asarray(Ws1, np.float32).astype(BF16)
    ws2 = np.asarray(Ws2, np.float32).astype(BF16)
    bs_in = np.asarray(bs, np.float32).reshape(SEC, 1)
    Wm2a = np.asarray(Wm2a, np.float32)
    Wm2b = np.asarray(Wm2b, np.float32)
    Wma = np.asarray(Wma, np.float32)
    Wmb = np.asarray(Wmb, np.float32)
    # pass-2 weights pre-scaled by 0.5*v (folds stack-mean + v-combine)
    wgsp = np.concatenate([
        0.5 * v1 * Wm2a[:SEC],
        0.5 * (v1 * Wm2a[SEC:] + v2 * Wma[F_T:]),
    ], axis=0).astype(BF16)
    was = (0.5 * v1 * Wm2b[:SEC]).astype(BF16)
    wxa = (0.5 * v2 * Wma[:F_T]).astype(BF16)
    wxb = (0.5 * v2 * Wmb[:F_T]).astype(BF16)
    wap = (0.5 * (v1 * Wm2b[SEC:] + v2 * Wmb[F_T:])).astype(BF16)
    bl = 0.5 * (v2 * np.asarray(bm, np.float32) + v1 * np.asarray(bm2, np.float32))
    blb = np.tile(bl.reshape(MC, 1), (1, 512)).astype(np.float32)

    in_maps = []
    for c in range(NCORES):
        r0, r1 = c * RL, (c + 1) * RL
        in_maps.append({
            "at": np.ascontiguousarray(A8[r0:r1].T),
            "xn": xn,
            "xtl": np.ascontiguousarray(xtf[:, r0:r1]).astype(BF16),
            "pt": np.ascontiguousarray(ptf[:, r0:r1]),
            "ws1": ws1, "ws2": ws2, "bs": bs_in,
            "wgsp": wgsp,
            "was": was, "wxa": wxa, "wxb": wxb, "wap": wap,
            "blb": blb,
        })

    kwargs = {}
    if trace_cores is not None:
        kwargs["trace_cores"] = trace_cores
    last_exc = None
    for attempt in range(3):
        try:
            res = run_bass_kernel_spmd(nc, in_maps, list(range(NCORES)),
                                       trace=trace, tmpdir=tmpdir, **kwargs)
            break
        except Exception as e:  # transient NRT device errors recover on retry
            last_exc = e
            import time as _time
            _time.sleep(3.0)
    else:
        raise last_exc
    out = np.concatenate([res.results[c]["out"] for c in range(NCORES)], axis=0)
    kernel.last_result = res
    return out

